# revision 1
# baseline (speedup 1.0000x reference)
"""Expert-parallel batched SwiGLU FFN for Trainium2 (8 NeuronCores, Bass/Tile).

Problem: out[e] = (silu(x[e] @ w1[e].T + b1[e]) * (x[e] @ w3[e].T + b3[e])) @ w2[e].T + b2[e]
with E=8, T=512, D_IN=7168, D_INT=2048, fp32.

Sharding: expert-parallel — core e owns expert e end-to-end, no communication.

Layout strategy: the TensorEngine contracts over the SBUF partition dim, so both
matmul operands need the contraction dim (d, then f) on partitions. DRAM-side we
stage every tensor pre-transposed on the host (free host-side rearrangement during
sharding, exactly how MoE frameworks store weights), so every device DMA is a
natural contiguous load and the kernel does zero on-chip transposes.

Numerics: matmuls run in float32r (fp32 storage, FP22 multiply, fp32 PSUM
accumulate) — full bf16-rate on the PE with ~1e-4 relative error.

Per-core schedule:
  phase 1 (h1t/h3t = w.T-contractions over d, output f-major):
    d-chunk outer loop (4 chunks x 14 subtiles) with SBUF partial accumulation
    so only one x-chunk is SBUF-resident at a time; per chunk, stream
    [128,7,512] half-tiles of w1t/w3t (one half per HWDGE queue - separate
    tiles give the Tile scheduler fine-grained DMA->matmul deps), 14-deep PSUM
    accumulation groups, DVE eviction into persistent partials. Last chunk
    fuses bias + silu + mul into gt (fp32r, in-place over the h3 partial).
  phase 2: out = gt.T @ w2t + b2, d-blocks of 512 as half-tiles across both
    queues (first blocks prefetched during phase 1's tail through the shared
    weight pool), 16-deep PSUM accumulation, DVE bias-add eviction, output
    DMAs split across both queues.

  The two HWDGE queues (nc.sync / nc.scalar) are kept byte-balanced and each
  first-needed tile sits at the head of one queue so the PE starts ~15us in.
"""

import numpy as np

import concourse.bacc as bacc
import concourse.mybir as mybir
import concourse.tile as tile
from concourse.bass_utils import run_bass_kernel_spmd

# Problem shape (hardcoded per contest contract).
E = 8
T = 512
D = 7168
F = 2048
P = 128

DO = D // P  # 56 d-subtiles
FO = F // P  # 16 f-subtiles
TT = T // P  # 4 t-subtiles

CH = 14  # d-subtiles per phase-1 chunk
NCH = DO // CH  # 4 chunks
DBW = 512  # phase-2 block width (d)
NDB = D // DBW  # 14 blocks

F32 = mybir.dt.float32
F32R = mybir.dt.float32r

_NC = None


def _emit(nc, tc, xt, w1t, w3t, w2t, b1, b3, b2r, out):
    add = mybir.AluOpType.add
    mult = mybir.AluOpType.mult
    silu = mybir.ActivationFunctionType.Silu

    xt_r = xt.ap().rearrange("(o p) t -> p o t", p=P)  # [P, DO, T]
    w1t_r = w1t.ap().rearrange("(o p) f -> p o f", p=P)  # [P, DO, F]
    w3t_r = w3t.ap().rearrange("(o p) f -> p o f", p=P)  # [P, DO, F]
    w2t_r = w2t.ap().rearrange("(o p) d -> p o d", p=P)  # [P, FO, D]
    b1_r = b1.ap().rearrange("(o p) -> p o", p=P)  # [P, FO]
    b3_r = b3.ap().rearrange("(o p) -> p o", p=P)  # [P, FO]
    out_r = out.ap().rearrange("(o p) d -> p o d", p=P)  # [P, TT, D]

    HCH = CH // 2  # 7: half-chunk (separate tiles => fine-grained DMA->MM deps)
    FBW1 = 512  # phase-1 weight block width (f)
    NFB1 = F // FBW1  # 4
    NFL = FBW1 // P  # 4 f-subtiles per block
    HFO = FO // 2  # 8: phase-2 w2 half block

    with tile.TileContext(nc) as tc:
        with (
            tc.tile_pool(name="persist", bufs=1) as persist,
            tc.tile_pool(name="wsp", bufs=5) as wsp,  # shared weight stream pool
        ):
            # h3 partial, finally overwritten in-place with gt (fp32r).
            h3p = persist.tile([P, FO, T], F32R, tag="h3p")
            b1s = persist.tile([P, FO], F32, tag="b1s")
            b3s = persist.tile([P, FO], F32, tag="b3s")
            def dma_wblk_half(w_r, c, fb, half, eng):
                """One [P, HCH, FBW1] half-block of a phase-1 weight tile."""
                wb = wsp.tile([P, HCH, FBW1], F32R, tag="wblk")
                lo = c * CH + half * HCH
                eng.dma_start(
                    wb[:], w_r[:, lo : lo + HCH, fb * FBW1 : (fb + 1) * FBW1]
                )
                return wb

            def dma_w2b_half(db, half, eng):
                """One [P, HFO, DBW] half of a phase-2 w2 block."""
                sl = slice(db * DBW, (db + 1) * DBW)
                w2h = wsp.tile([P, HFO, DBW], F32R, tag="wblk")
                fsl = slice(0, HFO) if half == 0 else slice(HFO, FO)
                eng.dma_start(w2h[:], w2t_r[:, fsl, sl])
                return w2h

            def dma_w2b(db):
                return (
                    dma_w2b_half(db, 0, nc.sync),
                    dma_w2b_half(db, 1, nc.scalar),
                )

            w2_prefetch = []

            # ---------------- phase 1 ----------------
            with (
                tc.tile_pool(name="p1", bufs=1) as p1,
                tc.tile_pool(name="xtp", bufs=3) as xtp,
                tc.tile_pool(name="s1p", bufs=2) as s1p,
                tc.tile_pool(name="ps1", bufs=6, space="PSUM") as psum1,
            ):
                h1p = p1.tile([P, FO, T], F32, tag="h1p")

                def dma_xt_half(c, half, eng):
                    xh = xtp.tile([P, HCH, T], F32R, tag="xt")
                    lo = c * CH + half * HCH
                    eng.dma_start(xh[:], xt_r[:, lo : lo + HCH, :])
                    return xh

                # chunk 0 / block 0: the first psum group (fl0, o0-6) needs
                # only xtA + a [P,7,128] sliver of w1; land those first, one
                # per queue, so the PE starts ~12us in. The rest of block 0
                # arrives in f/o-sliced pieces right behind them.
                xt0a = dma_xt_half(0, 0, nc.sync)
                wq_a = wsp.tile([P, HCH, P], F32R, tag="wblk")
                nc.scalar.dma_start(wq_a[:], w1t_r[:, :HCH, :P])
                xt0b = dma_xt_half(0, 1, nc.scalar)
                wq_c = dma_wblk_half(w1t_r, 0, 0, 1, nc.sync)  # o 7-13, all fl
                wq_b = wsp.tile([P, HCH, FBW1 - P], F32R, tag="wblk")
                nc.scalar.dma_start(wq_b[:], w1t_r[:, :HCH, P:FBW1])
                xt_next = (xt0a, xt0b)
                for c in range(NCH):
                    xt_ab = xt_next
                    for fb in range(NFB1):
                        if c == 1 and fb == 0:
                            nc.sync.dma_start(b1s[:], b1_r)
                            nc.sync.dma_start(b3s[:], b3_r)
                        if c + 1 < NCH:
                            # spread next chunk's xt, one half per queue
                            if fb == 1:
                                xa = dma_xt_half(c + 1, 0, nc.sync)
                            elif fb == 2:
                                xt_next = (xa, dma_xt_half(c + 1, 1, nc.scalar))
                        s1_tiles = {}
                        for wi, w_r in ((0, w1t_r), (1, w3t_r)):
                            first_block = c == 0 and fb == 0 and wi == 0
                            if not first_block:
                                wba = dma_wblk_half(w_r, c, fb, 0, nc.sync)
                                wbb = dma_wblk_half(w_r, c, fb, 1, nc.scalar)
                            for fl in range(NFL):
                                ft = fb * NFL + fl
                                ps = psum1.tile([P, T], F32, tag="ps")
                                for o in range(CH):
                                    half, ol = divmod(o, HCH)
                                    if first_block:
                                        if half == 1:
                                            lhsT = wq_c[:, ol, fl * P : (fl + 1) * P]
                                        elif fl == 0:
                                            lhsT = wq_a[:, ol, :]
                                        else:
                                            lhsT = wq_b[:, ol, (fl - 1) * P : fl * P]
                                    else:
                                        wb = wba if half == 0 else wbb
                                        lhsT = wb[:, ol, fl * P : (fl + 1) * P]
                                    xh = xt_ab[half]
                                    nc.tensor.matmul(
                                        ps[:],
                                        lhsT,
                                        xh[:, ol, :],
                                        start=(o == 0),
                                        stop=(o == CH - 1),
                                    )
                                hp = h1p if wi == 0 else h3p
                                if c == 0:
                                    nc.vector.tensor_copy(hp[:, ft, :], ps[:])
                                elif c < NCH - 1:
                                    nc.vector.tensor_add(
                                        hp[:, ft, :], hp[:, ft, :], ps[:]
                                    )
                                elif wi == 0:
                                    # h1 complete: s1 = silu(h1 + b1)
                                    nc.vector.tensor_add(
                                        hp[:, ft, :], hp[:, ft, :], ps[:]
                                    )
                                    s1_t = s1p.tile([P, T], F32, tag="s1")
                                    nc.scalar.activation(
                                        s1_t[:],
                                        hp[:, ft, :],
                                        silu,
                                        bias=b1s[:, ft : ft + 1],
                                    )
                                    s1_tiles[fl] = s1_t
                                else:
                                    # h3 complete: gt = (h3 + b3) * s1, fp32r,
                                    # written in place over the h3 partial.
                                    nc.vector.tensor_add(
                                        hp[:, ft, :], hp[:, ft, :], ps[:]
                                    )
                                    nc.vector.scalar_tensor_tensor(
                                        out=hp[:, ft, :],
                                        in0=hp[:, ft, :],
                                        scalar=b3s[:, ft : ft + 1],
                                        in1=s1_tiles[fl][:],
                                        op0=add,
                                        op1=mult,
                                    )
                        if c == NCH - 1 and fb == 2:
                            # phase-2 head start: db0 A-half early
                            w2_pre_a0 = dma_w2b_half(0, 0, nc.sync)
                    if c == NCH - 1:
                        # rest of the phase-2 head start
                        w2_prefetch.append(
                            (w2_pre_a0, dma_w2b_half(0, 1, nc.scalar))
                        )
                        w2_prefetch.append(
                            (
                                dma_w2b_half(1, 0, nc.sync),
                                dma_w2b_half(1, 1, nc.scalar),
                            )
                        )

            gt = h3p  # [P, FO, T] fp32r

            # ---------------- phase 2 ----------------
            with (
                tc.tile_pool(name="b2p", bufs=2) as b2p,
                tc.tile_pool(name="osp", bufs=2) as osp,
                tc.tile_pool(name="ps2", bufs=6, space="PSUM") as psum2,
            ):
                for db in range(NDB):
                    if db < len(w2_prefetch):
                        w2a, w2b_ = w2_prefetch[db]
                    else:
                        w2a, w2b_ = dma_w2b(db)
                    b2sl = b2p.tile([P, DBW], F32, tag="b2sl")
                    nc.scalar.dma_start(
                        b2sl[:], b2r.ap()[:, db * DBW : (db + 1) * DBW]
                    )
                    ost = osp.tile([P, TT, DBW], F32, tag="ost")
                    for tt in range(TT):
                        ps = psum2.tile([P, DBW], F32, tag="ps2")
                        for fo in range(FO):
                            w2h = w2a if fo < HFO else w2b_
                            nc.tensor.matmul(
                                ps[:],
                                gt[:, fo, tt * P : (tt + 1) * P],
                                w2h[:, fo % HFO, :],
                                start=(fo == 0),
                                stop=(fo == FO - 1),
                            )
                        nc.vector.tensor_add(
                            ost[:, tt, :],
                            ps[:],
                            b2sl[:],
                        )
                        if db == NDB - 1:
                            # stream the final block out per t-subtile
                            eng = nc.sync if tt % 2 == 0 else nc.scalar
                            eng.dma_start(
                                out_r[:, tt, db * DBW : (db + 1) * DBW],
                                ost[:, tt, :],
                            )
                    if db < NDB - 1:
                        dsl = slice(db * DBW, (db + 1) * DBW)
                        nc.sync.dma_start(out_r[:, :2, dsl], ost[:, :2, :])
                        nc.scalar.dma_start(out_r[:, 2:, dsl], ost[:, 2:, :])


def build():
    global _NC
    if _NC is not None:
        return _NC
    nc = bacc.Bacc("TRN2", target_bir_lowering=False, debug=False, num_devices=E)
    xt = nc.dram_tensor("xt", [D, T], F32R, kind="ExternalInput")
    w1t = nc.dram_tensor("w1t", [D, F], F32R, kind="ExternalInput")
    w3t = nc.dram_tensor("w3t", [D, F], F32R, kind="ExternalInput")
    w2t = nc.dram_tensor("w2t", [F, D], F32R, kind="ExternalInput")
    b1 = nc.dram_tensor("b1", [F], F32, kind="ExternalInput")
    b3 = nc.dram_tensor("b3", [F], F32, kind="ExternalInput")
    b2r = nc.dram_tensor("b2r", [P, D], F32, kind="ExternalInput")
    out = nc.dram_tensor("out", [T, D], F32, kind="ExternalOutput")
    _emit(nc, None, xt, w1t, w3t, w2t, b1, b3, b2r, out)
    nc.compile()
    _NC = nc
    return nc


def make_in_maps(x, w1, b1, w3, b3, w2, b2):
    x = np.asarray(x, dtype=np.float32)
    w1 = np.asarray(w1, dtype=np.float32)
    b1 = np.asarray(b1, dtype=np.float32)
    w3 = np.asarray(w3, dtype=np.float32)
    b3 = np.asarray(b3, dtype=np.float32)
    w2 = np.asarray(w2, dtype=np.float32)
    b2 = np.asarray(b2, dtype=np.float32)
    in_maps = []
    for e in range(E):
        in_maps.append(
            {
                "xt": np.ascontiguousarray(x[e].T),  # [D, T]
                "w1t": np.ascontiguousarray(w1[e].T),  # [D, F]
                "w3t": np.ascontiguousarray(w3[e].T),  # [D, F]
                "w2t": np.ascontiguousarray(w2[e].T),  # [F, D]
                "b1": b1[e],
                "b3": b3[e],
                "b2r": np.ascontiguousarray(
                    np.broadcast_to(b2[e], (P, D))
                ),  # [P, D]
            }
        )
    return in_maps


def run(x, w1, b1, w3, b3, w2, b2, **spmd_kwargs):
    nc = build()
    in_maps = make_in_maps(x, w1, b1, w3, b3, w2, b2)
    res = run_bass_kernel_spmd(nc, in_maps, core_ids=list(range(E)), **spmd_kwargs)
    out = np.stack([res.results[e]["out"] for e in range(E)], axis=0)
    return out, res


def kernel(x, w1, b1, w3, b3, w2, b2):
    out, _ = run(x, w1, b1, w3, b3, w2, b2)
    return out



# revision 9
# speedup vs baseline: 1.1276x; 1.1276x over previous
"""Expert-parallel batched SwiGLU FFN for Trainium2 (8 NeuronCores, Bass/Tile).

Problem: out[e] = (silu(x[e] @ w1[e].T + b1[e]) * (x[e] @ w3[e].T + b3[e])) @ w2[e].T + b2[e]
with E=8, T=512, D_IN=7168, D_INT=2048, fp32 reference.

Sharding: expert-parallel - core e owns expert e end-to-end, no communication.

v2 strategy (from the v1 trace): v1 ran fp32r everywhere, putting HBM traffic
(209 MB/core = 584 us at 358 GB/s) right on top of the PE roofline (2688
512-col matmuls = 580 us), so every DMA hiccup was a PE stall (39 us of gaps +
7 HAM re-throttles). v2 stages x/w1/w3/w2 as bf16 (halves DMA to ~114 MB =
~320 us, same PE rate, rel err ~5e-4 -> ~4e-3, well under the 2e-2 gate) so the
PE is the sole critical engine.

Layouts are host-swizzled so every DMA is per-partition contiguous:
  xs  [128][o][t]        o = d//128 (56), partition = d%128
  w1s/w3s [128][ft][o][fl]  ft = f//128 (16)
  w2s [128][db][fo][dw]  db = d//1024 (7), fo = f//128, dw = d%1024

Per-core schedule:
  phase 1: x fully SBUF-resident (56 KiB/part bf16). For each (ft, w in
    {w1,w3}): one 56-deep PSUM accumulation group (stationary = w tile
    [128d,128f], moving = x [128d,512t]); silu+bias straight off PSUM on
    ScalarE, then gt[ft] = (h3+b3)*s1 on DVE, written bf16.
    Startup: the ft0+ft1 group quadruple is interleaved chunk-by-chunk with
    the x DMA stream (8 chunks of 7 d-subtiles) so the PE starts ~3.5 us in
    instead of waiting ~20 us for all of x.
  phase 2: out[t,d] = sum_f gt[f,t]*w2[f,d]. gt tiles stationary, w2 moving,
    16-deep PSUM groups over fo into [128,1024] (2-bank) tiles per (db,ts);
    DVE adds b2 (host-broadcast) and output DMAs stream out per (db,ts).
    w2 blocks ride queue-S only, output DMAs queue-A only, so a ring-gated
    weight DMA can never head-of-line-block an output transfer.
"""

import numpy as np
import ml_dtypes

import concourse.bacc as bacc
import concourse.mybir as mybir
import concourse.tile as tile
from concourse.bass_utils import run_bass_kernel_spmd

# Problem shape (hardcoded per contest contract).
E = 8
T = 512
D = 7168
F = 2048
P = 128

DO = D // P  # 56 d-subtiles
FO = F // P  # 16 f-subtiles
TT = T // P  # 4 t-subtiles

XC = 8  # x chunks (phase-1 startup granularity)
XCW = DO // XC  # 7 d-subtiles per x chunk
NPRE = 2  # ft groups interleaved with the x stream at startup
HW_ = 28  # phase-1 steady-state weight half-block width (d-subtiles)
DBW = 1024  # phase-2 block width (d)
NDB = D // DBW  # 7 blocks
HFO = FO // 2  # 8: phase-2 w2 half block (fo)

F32 = mybir.dt.float32
BF16 = mybir.dt.bfloat16
BF = ml_dtypes.bfloat16

_NC = None


def _emit(nc, xs, w1s, w3s, w2s, b1, b3, b2r, out):
    add = mybir.AluOpType.add
    mult = mybir.AluOpType.mult
    silu = mybir.ActivationFunctionType.Silu

    xs_r = xs.ap().rearrange("p (o t) -> p o t", o=DO)  # [P, DO, T]
    w1s_r = w1s.ap().rearrange("p (ft x) -> p ft x", ft=FO)  # [P, FO, DO*P]
    w3s_r = w3s.ap().rearrange("p (ft x) -> p ft x", ft=FO)
    w2s_r = w2s.ap().rearrange("p (db x) -> p db x", db=NDB)  # [P, NDB, FO*DBW]
    b1_r = b1.ap().rearrange("(o p) -> p o", p=P)  # [P, FO]
    b3_r = b3.ap().rearrange("(o p) -> p o", p=P)
    out_r = out.ap().rearrange("(o p) d -> p o d", p=P)  # [P, TT, D]

    Q = [nc.sync, nc.scalar]
    qi = [0]

    def nextq():
        e = Q[qi[0] & 1]
        qi[0] += 1
        return e

    with tile.TileContext(nc) as tc:
        with (
            tc.tile_pool(name="persist", bufs=1) as persist,
            tc.tile_pool(name="wsp", bufs=1) as wsp,
            tc.tile_pool(name="evp", bufs=1) as evp,
            tc.tile_pool(name="psp", bufs=1, space="PSUM") as psp,
        ):
            gt = persist.tile([P, FO, T], BF16, tag="gt")
            b2s = persist.tile([P, D], F32, tag="b2s")
            b1s = persist.tile([P, FO], F32, tag="b1s")
            b3s = persist.tile([P, FO], F32, tag="b3s")
            xch = [
                persist.tile([P, XCW, T], BF16, tag=f"xc{c}", name=f"xc{c}")
                for c in range(XC)
            ]

            # ---- phase-1 startup DMA stream: x chunks + ft0/ft1 w quarters,
            # interleaved so the PE can chase the stream chunk by chunk. The
            # wq ring is deep (12 = 3 chunks of lookahead) so ring-gated
            # quarters can't convoy-block later x chunks in queue order.
            wq = {}  # (wi, ft, c) -> quarter tile [P, XCW*P]
            pre = [(0, 0), (1, 0), (0, 1), (1, 1)]  # (wi, ft) group order
            for c in range(XC):
                if c == 0:
                    # sliver split: first 2 d-subtiles land ~1.5us sooner so
                    # the first matmul isn't gated on the whole chunk
                    nc.sync.dma_start(xch[0][:, :2, :], xs_r[:, :2, :])
                    nc.scalar.dma_start(xch[0][:, 2:, :], xs_r[:, 2:XCW, :])
                else:
                    nextq().dma_start(
                        xch[c][:], xs_r[:, c * XCW : (c + 1) * XCW, :]
                    )
                for wi, ft in pre:
                    wsrc = w1s_r if wi == 0 else w3s_r
                    q = wsp.tile([P, XCW * P], BF16, tag="wq", bufs=11, name="wq")
                    nextq().dma_start(
                        q[:], wsrc[:, ft, c * XCW * P : (c + 1) * XCW * P]
                    )
                    wq[(wi, ft, c)] = q
            # biases are tiny (8 KB); load them here — their consumers (the
            # part-A evictions) are emitted below, so the write must precede
            # them in program order for Tile to sequence it.
            nc.sync.dma_start(b1s[:], b1_r)
            nc.scalar.dma_start(b3s[:], b3_r)

            # ---- phase-1 startup matmuls: 4 groups interleaved chunk-wise.
            ps_pre = {}
            for wi, ft in pre:
                ps_pre[(wi, ft)] = psp.tile(
                    [P, T], F32, tag="ps1", bufs=4, name="ps1"
                )
            for c in range(XC):
                for wi, ft in pre:
                    q = wq[(wi, ft, c)]
                    for j in range(XCW):
                        nc.tensor.matmul(
                            ps_pre[(wi, ft)][:],
                            q[:, j * P : (j + 1) * P],
                            xch[c][:, j, :],
                            start=(c == 0 and j == 0),
                            stop=(c == XC - 1 and j == XCW - 1),
                        )

            s1_cur = {}  # ft -> s1 tile

            def evict_p1(wi, ft, ps):
                if wi == 0:
                    s1 = evp.tile([P, T], F32, tag="s1", bufs=2, name="s1")
                    nc.scalar.activation(
                        s1[:], ps[:], silu, bias=b1s[:, ft : ft + 1]
                    )
                    s1_cur[ft] = s1
                else:
                    nc.vector.scalar_tensor_tensor(
                        out=gt[:, ft, :],
                        in0=ps[:],
                        scalar=b3s[:, ft : ft + 1],
                        in1=s1_cur[ft][:],
                        op0=add,
                        op1=mult,
                    )

            for wi, ft in pre:
                evict_p1(wi, ft, ps_pre[(wi, ft)])

            # ---- phase-1 steady state: ft 2..15, full 56-deep groups.
            # ft2/ft3 halves ride directly behind the part-A stream so they
            # land before the PE drains part A; biases after them; b2s later.
            halves = {}
            for ft in range(NPRE, FO):
                for wi in range(2):
                    wsrc = w1s_r if wi == 0 else w3s_r
                    hA = wsp.tile([P, HW_ * P], BF16, tag="wh", bufs=4, name="whA")
                    hB = wsp.tile([P, HW_ * P], BF16, tag="wh", bufs=4, name="whB")
                    nextq().dma_start(hA[:], wsrc[:, ft, : HW_ * P])
                    nextq().dma_start(hB[:], wsrc[:, ft, HW_ * P :])
                    halves[(wi, ft)] = (hA, hB)
                if ft == 5:
                    nc.sync.dma_start(b2s[:, : D // 2], b2r.ap()[:, : D // 2])
                    nc.scalar.dma_start(b2s[:, D // 2 :], b2r.ap()[:, D // 2 :])

            for ft in range(NPRE, FO):
                for wi in range(2):
                    hA, hB = halves[(wi, ft)]
                    ps = psp.tile([P, T], F32, tag="ps1", bufs=4, name="ps1")
                    for o in range(DO):
                        if o < HW_:
                            lhsT = hA[:, o * P : (o + 1) * P]
                        else:
                            lhsT = hB[:, (o - HW_) * P : (o - HW_ + 1) * P]
                        nc.tensor.matmul(
                            ps[:],
                            lhsT,
                            xch[o // XCW][:, o % XCW, :],
                            start=(o == 0),
                            stop=(o == DO - 1),
                        )
                    evict_p1(wi, ft, ps)

            # ---------------- phase 2 ----------------
            # w2 blocks stream on queue-S only; output DMAs on queue-A only.
            w2h = {}
            for db in range(NDB):
                hA = wsp.tile([P, HFO * DBW], BF16, tag="w2h", bufs=3, name="w2hA")
                hB = wsp.tile([P, HFO * DBW], BF16, tag="w2h", bufs=3, name="w2hB")
                nc.sync.dma_start(hA[:], w2s_r[:, db, : HFO * DBW])
                nc.sync.dma_start(hB[:], w2s_r[:, db, HFO * DBW :])
                w2h[db] = (hA, hB)

            for db in range(NDB):
                hA, hB = w2h[db]
                dsl = slice(db * DBW, (db + 1) * DBW)
                for ts in range(TT):
                    ps2 = psp.tile([P, DBW], F32, tag="ps2", bufs=2, name="ps2")
                    for fo in range(FO):
                        wh = hA if fo < HFO else hB
                        base = (fo % HFO) * DBW
                        lhsT = gt[:, fo, ts * P : (ts + 1) * P]
                        nc.tensor.matmul(
                            ps2[:, :512],
                            lhsT,
                            wh[:, base : base + 512],
                            start=(fo == 0),
                            stop=(fo == FO - 1),
                        )
                        nc.tensor.matmul(
                            ps2[:, 512:],
                            lhsT,
                            wh[:, base + 512 : base + DBW],
                            start=(fo == 0),
                            stop=(fo == FO - 1),
                        )
                    ost = evp.tile([P, DBW], F32, tag="ost", bufs=2, name="ost")
                    if db == NDB - 1:
                        # tail: split eviction + output across both queues so
                        # the last bytes leave ~2us sooner
                        h = DBW // 2
                        nc.vector.tensor_add(
                            ost[:, :h], ps2[:, :h], b2s[:, db * DBW : db * DBW + h]
                        )
                        nc.sync.dma_start(
                            out_r[:, ts, db * DBW : db * DBW + h], ost[:, :h]
                        )
                        nc.vector.tensor_add(
                            ost[:, h:], ps2[:, h:], b2s[:, db * DBW + h : (db + 1) * DBW]
                        )
                        nc.scalar.dma_start(
                            out_r[:, ts, db * DBW + h : (db + 1) * DBW], ost[:, h:]
                        )
                    else:
                        nc.vector.tensor_add(ost[:], ps2[:], b2s[:, dsl])
                        nc.scalar.dma_start(out_r[:, ts, dsl], ost[:])


def build():
    global _NC
    if _NC is not None:
        return _NC
    nc = bacc.Bacc("TRN2", target_bir_lowering=False, debug=False, num_devices=E)
    xs = nc.dram_tensor("xs", [P, DO * T], BF16, kind="ExternalInput")
    w1s = nc.dram_tensor("w1s", [P, FO * DO * P], BF16, kind="ExternalInput")
    w3s = nc.dram_tensor("w3s", [P, FO * DO * P], BF16, kind="ExternalInput")
    w2s = nc.dram_tensor("w2s", [P, NDB * FO * DBW], BF16, kind="ExternalInput")
    b1 = nc.dram_tensor("b1", [F], F32, kind="ExternalInput")
    b3 = nc.dram_tensor("b3", [F], F32, kind="ExternalInput")
    b2r = nc.dram_tensor("b2r", [P, D], F32, kind="ExternalInput")
    out = nc.dram_tensor("out", [T, D], F32, kind="ExternalOutput")
    _emit(nc, xs, w1s, w3s, w2s, b1, b3, b2r, out)
    nc.compile()
    _NC = nc
    return nc


def make_in_maps(x, w1, b1, w3, b3, w2, b2):
    x = np.asarray(x, dtype=np.float32)
    w1 = np.asarray(w1, dtype=np.float32)
    b1 = np.asarray(b1, dtype=np.float32)
    w3 = np.asarray(w3, dtype=np.float32)
    b3 = np.asarray(b3, dtype=np.float32)
    w2 = np.asarray(w2, dtype=np.float32)
    b2 = np.asarray(b2, dtype=np.float32)
    in_maps = []
    for e in range(E):
        # xs[p][o][t] = x[e][t, o*128+p]
        xs = x[e].reshape(T, DO, P).transpose(2, 1, 0).reshape(P, -1).astype(BF)
        # w1s[p][ft][o][fl] = w1[e][ft*128+fl, o*128+p]
        w1s = (
            w1[e].reshape(FO, P, DO, P).transpose(3, 0, 2, 1).reshape(P, -1).astype(BF)
        )
        w3s = (
            w3[e].reshape(FO, P, DO, P).transpose(3, 0, 2, 1).reshape(P, -1).astype(BF)
        )
        # w2s[p][db][fo][dw] = w2[e][db*1024+dw, fo*128+p]
        w2s = (
            w2[e]
            .reshape(NDB, DBW, FO, P)
            .transpose(3, 0, 2, 1)
            .reshape(P, -1)
            .astype(BF)
        )
        in_maps.append(
            {
                "xs": xs,
                "w1s": w1s,
                "w3s": w3s,
                "w2s": w2s,
                "b1": b1[e],
                "b3": b3[e],
                "b2r": np.ascontiguousarray(np.broadcast_to(b2[e], (P, D))),
            }
        )
    return in_maps


def run(x, w1, b1, w3, b3, w2, b2, **spmd_kwargs):
    nc = build()
    in_maps = make_in_maps(x, w1, b1, w3, b3, w2, b2)
    res = run_bass_kernel_spmd(nc, in_maps, core_ids=list(range(E)), **spmd_kwargs)
    out = np.stack([res.results[e]["out"] for e in range(E)], axis=0)
    return out, res


def kernel(x, w1, b1, w3, b3, w2, b2):
    out, _ = run(x, w1, b1, w3, b3, w2, b2)
    return out
